# revision 1
# baseline (speedup 1.0000x reference)
"""Trainium2 Bass kernel for relational GNN message passing (BlockDecomposition).

Strategy (8 NeuronCores, SPMD, no collectives):
  - Symmetrize edges into 1.6M directed messages + 50k self-loop messages.
  - Shard messages by destination node range (6250 nodes/core); each core
    produces its own output rows.
  - Host sorts each core's messages by (dest bucket of 128 nodes, src half,
    relation). dma_gather int16 indices only address 32768 rows, so each
    bucket's slots are laid out as [low-half region | high-half region]; each
    region is one dma_gather (low from x[0:32768], high from x[32768:]).
    The gather ucode requires every index position to be valid (interior
    skips corrupt), so pad slots gather dummy row 0 and carry weight 0.
  - Per (bucket, rel) group: static tile counts T_LO/T_HI = max over cores
    (same compiled program for all 8 cores; slack slots are dummies).
  - Device, per bucket:
      * two dma_gathers of x[src] rows into G,
      * DVE: G *= w (dummy slots have w = 0),
      * per rel: aggregation matmuls aggT += G_tile^T @ H_tile in PSUM, with
        one-hot H[e, n] = (dst_row[e] == n) built by DVE is_equal vs iota
        (dummy slots have dst_row = 255 -> zero column),
      * transform matmuls outT += BW_r^T @ aggT accumulated over 9 relations,
      * store the outT tile (128 dims x 128 nodes) to HBM.
  - Host transposes/concatenates per-core outputs.
"""

import sys

import numpy as np

sys.path.insert(0, "/opt/trn_rl_repo")

# Problem constants (hardcoded per spec).
N_NODES = 50000
DIM = 128
N_REL = 8  # edge relations; +1 self-loop "relation"
NCORES = 8
SHARD = N_NODES // NCORES  # 6250
NBUCK = (SHARD + 127) // 128  # 49
PADN = NBUCK * 128  # 6272
LO = 32768  # int16-addressable rows per gather table

_PAD_ROW = 255.0  # no is_equal match vs iota 0..127 -> zero one-hot column

_cache = {}
last_result = None  # BassKernelResults of the most recent run (for profiling)


def _np_dt(dt_name):
    if dt_name == "float32":
        return np.float32
    import ml_dtypes

    return np.dtype(getattr(ml_dtypes, dt_name))


class Layout:
    """Static slot layout shared by all cores (max tile counts over cores)."""

    def __init__(self, tlo, thi):
        # tlo/thi: [NBUCK, N_REL+1] tiles per (bucket, rel) per src half
        self.tlo = tlo
        self.thi = thi
        self.nlo = tlo.sum(axis=1)  # lo-region tiles per bucket
        self.nhi = thi.sum(axis=1)
        self.tb = self.nlo + self.nhi  # tiles per bucket
        self.c0 = np.zeros(NBUCK, dtype=np.int64)  # global tile offset
        self.c0[1:] = np.cumsum(self.tb)[:-1]
        self.nt = int(self.tb.sum())  # total slot tiles per core
        # within-bucket tile offsets of each (rel, half) group part
        self.lo_off = np.zeros((NBUCK, N_REL + 1), dtype=np.int64)
        self.lo_off[:, 1:] = np.cumsum(tlo, axis=1)[:, :-1]
        self.hi_off = np.zeros((NBUCK, N_REL + 1), dtype=np.int64)
        self.hi_off[:, 1:] = np.cumsum(thi, axis=1)[:, :-1]
        # idx-array column offsets per bucket (NI/16 columns per bucket)
        self.olo = np.zeros(NBUCK, dtype=np.int64)
        self.olo[1:] = np.cumsum(self.nlo * 8)[:-1]
        self.ohi = np.zeros(NBUCK, dtype=np.int64)
        self.ohi[1:] = np.cumsum(self.nhi * 8)[:-1]
        self.ilo_cols = int((self.nlo * 8).sum())
        self.ihi_cols = int((self.nhi * 8).sum())

    def key(self):
        return (self.tlo.tobytes(), self.thi.tobytes())


def _message_arrays(src, dst, rel, w, keep, k):
    """All directed messages destined to core k, plus its self-loops."""
    m = (dst >= k * SHARD) & (dst < (k + 1) * SHARD)
    n0 = k * SHARD
    s_k = np.concatenate([src[m], np.arange(n0, n0 + SHARD, dtype=np.int64)])
    l_k = np.concatenate([dst[m] - n0, np.arange(SHARD, dtype=np.int64)])
    r_k = np.concatenate([rel[m], np.full(SHARD, N_REL, dtype=np.int64)])
    w_k = np.concatenate([w[m], keep[n0 : n0 + SHARD].astype(np.float32)])
    return s_k, l_k, r_k, w_k


def _prepare_layout(src, dst, rel, w, keep):
    cnt = np.zeros((NCORES, NBUCK, N_REL + 1, 2), dtype=np.int64)
    percore = []
    for k in range(NCORES):
        s_k, l_k, r_k, w_k = _message_arrays(src, dst, rel, w, keep, k)
        half = (s_k >= LO).astype(np.int64)
        bucket = l_k // 128
        np.add.at(cnt[k], (bucket, r_k, half), 1)
        percore.append((s_k, l_k, r_k, w_k, half, bucket))
    tlo = -(-cnt[:, :, :, 0].max(axis=0) // 128)
    thi = -(-cnt[:, :, :, 1].max(axis=0) // 128)
    return Layout(tlo, thi), percore


def _prepare_core_meta(lay, percore, dt_name):
    npdt = _np_dt(dt_name)
    ilo_all, ihi_all, grow_all, gw_all = [], [], [], []
    for k in range(NCORES):
        s_k, l_k, r_k, w_k, half, bucket = percore[k]
        row = l_k % 128
        # sort by src within each (bucket, rel, half) group: monotonic gather
        # addresses per instruction window -> HBM row locality
        order = np.lexsort((s_k, half, r_k, bucket))
        s_k, r_k, w_k, half, bucket, row = (
            a[order] for a in (s_k, r_k, w_k, half, bucket, row)
        )
        g = (bucket * (N_REL + 1) + r_k) * 2 + half
        sizes = np.bincount(g, minlength=NBUCK * (N_REL + 1) * 2)
        starts = np.zeros_like(sizes)
        starts[1:] = np.cumsum(sizes)[:-1]
        rank = np.arange(len(g)) - starts[g]

        # global tile of each message
        t_lo = lay.c0[bucket] + lay.lo_off[bucket, r_k] + rank // 128
        t_hi = lay.c0[bucket] + lay.nlo[bucket] + lay.hi_off[bucket, r_k] + rank // 128
        t = np.where(half == 0, t_lo, t_hi)
        p = rank % 128

        grow = np.full((128, lay.nt), _PAD_ROW, dtype=np.float32)
        gw = np.zeros((128, lay.nt), dtype=np.float32)
        grow[p, t] = row
        gw[p, t] = w_k

        # gather-position i within the bucket's region; idx element i of
        # bucket b lives at [i % 16, region_col_offset_b + i // 16]
        t_in_region = np.where(
            half == 0, t - lay.c0[bucket], t - lay.c0[bucket] - lay.nlo[bucket]
        )
        i_pos = t_in_region * 128 + p
        col_off = np.where(half == 0, lay.olo[bucket], lay.ohi[bucket])
        cols = col_off + i_pos // 16
        prow = i_pos % 16
        ilo = np.zeros((16, lay.ilo_cols), dtype=np.int16)  # dummies -> row 0
        ihi = np.zeros((16, lay.ihi_cols), dtype=np.int16)
        is_lo = half == 0
        ilo[prow[is_lo], cols[is_lo]] = s_k[is_lo].astype(np.int16)
        ihi[prow[~is_lo], cols[~is_lo]] = (s_k[~is_lo] - LO).astype(np.int16)

        ilo_all.append(np.tile(ilo, (8, 1)))
        ihi_all.append(np.tile(ihi, (8, 1)))
        grow_all.append(grow.astype(npdt))
        gw_all.append(gw.astype(npdt))
    return ilo_all, ihi_all, grow_all, gw_all


def _build_program(dt_name, lay, repeat=1):
    """Build + compile the Bass/Tile program. Returns (nc, io_names).

    repeat > 1 wraps the compute in a hardware loop re-running the identical
    body (same output) -- used for differential wall-clock timing.
    """
    from contextlib import ExitStack

    from concourse import bacc, mybir
    import concourse.tile as tile

    DT = getattr(mybir.dt, dt_name)
    f32 = mybir.dt.float32
    i16 = mybir.dt.int16

    nc = bacc.Bacc(None, target_bir_lowering=False, debug=False)

    with tile.TileContext(nc) as tc:
        with tc.tile_pool(name="dram", bufs=1, space="DRAM") as dram:
            xt_d = dram.tile([N_NODES, DIM], DT, kind="ExternalInput", name="xt")
            bw_d = dram.tile([128, (N_REL + 1) * 128], DT, kind="ExternalInput", name="bw")
            iota_d = dram.tile([128, 128], DT, kind="ExternalInput", name="iota")
            ilo_d = dram.tile([128, lay.ilo_cols], i16, kind="ExternalInput", name="ilo")
            ihi_d = dram.tile([128, lay.ihi_cols], i16, kind="ExternalInput", name="ihi")
            grow_d = dram.tile([128, lay.nt], DT, kind="ExternalInput", name="grow")
            gw_d = dram.tile([128, lay.nt], DT, kind="ExternalInput", name="gw")
            out_d = dram.tile([128, PADN], f32, kind="ExternalOutput", name="outT")

            max_tb = int(lay.tb.max())
            with (
                tc.tile_pool(name="const", bufs=1) as constp,
                tc.tile_pool(name="ipool", bufs=3) as ipool,
                tc.tile_pool(name="gpool", bufs=3) as gpool,
                tc.tile_pool(name="hpool", bufs=6) as hpool,
                tc.tile_pool(name="aggsb", bufs=6) as aggsbp,
                tc.tile_pool(name="outsb", bufs=3) as outsbp,
                tc.tile_pool(name="aggps", bufs=4, space="PSUM") as aggpsp,
                tc.tile_pool(name="outps", bufs=2, space="PSUM") as outpsp,
            ):
                iota_s = constp.tile([128, 128], DT)
                bw_s = constp.tile([128, (N_REL + 1) * 128], DT)
                grow_s = constp.tile([128, lay.nt], DT)
                gw_s = constp.tile([128, lay.nt], DT)
                nc.sync.dma_start(out=iota_s[:], in_=iota_d[:])
                nc.sync.dma_start(out=bw_s[:], in_=bw_d[:])
                nc.sync.dma_start(out=grow_s[:], in_=grow_d[:])
                nc.sync.dma_start(out=gw_s[:], in_=gw_d[:])

                rep_ctx = ExitStack()
                if repeat > 1:
                    rep_ctx.enter_context(tc.For_i(0, repeat, 1))
                for b in range(NBUCK):
                    c0 = int(lay.c0[b])
                    nlo = int(lay.nlo[b])
                    nhi = int(lay.nhi[b])
                    tb = nlo + nhi
                    ilo = ipool.tile([128, 8 * max_tb], i16, name="ilo_t")
                    ihi = ipool.tile([128, 8 * max_tb], i16, name="ihi_t")
                    nc.sync.dma_start(
                        out=ilo[:, : 8 * nlo],
                        in_=ilo_d[:, int(lay.olo[b]) : int(lay.olo[b]) + 8 * nlo],
                    )
                    nc.sync.dma_start(
                        out=ihi[:, : 8 * nhi],
                        in_=ihi_d[:, int(lay.ohi[b]) : int(lay.ohi[b]) + 8 * nhi],
                    )
                    G = gpool.tile([128, max_tb, DIM], DT, name="G")
                    nc.gpsimd.dma_gather(
                        G[:, 0:nlo, :],
                        xt_d[0:LO],
                        ilo[:, : 8 * nlo],
                        nlo * 128,
                        nlo * 128,
                        DIM,
                        single_packet=False,
                    )
                    nc.gpsimd.dma_gather(
                        G[:, nlo:tb, :],
                        xt_d[LO:N_NODES],
                        ihi[:, : 8 * nhi],
                        nhi * 128,
                        nhi * 128,
                        DIM,
                        single_packet=False,
                    )
                    # G *= w  (broadcast along features; dummy slots have w=0)
                    nc.vector.tensor_tensor(
                        out=G[:, 0:tb, :],
                        in0=G[:, 0:tb, :],
                        in1=gw_s[:, c0 : c0 + tb].unsqueeze(2).broadcast_to([128, tb, DIM]),
                        op=mybir.AluOpType.mult,
                    )

                    out_ps = outpsp.tile([128, 128], f32, name="out_ps", space="PSUM")
                    for r in range(N_REL + 1):
                        tl = int(lay.tlo[b, r])
                        th = int(lay.thi[b, r])
                        T = tl + th
                        # group's global tile columns within G / grow
                        g_lo = int(lay.lo_off[b, r])
                        g_hi = nlo + int(lay.hi_off[b, r])
                        H = hpool.tile([128, max(T, 1), 128], DT, name="H", tag="H")
                        if tl:
                            nc.vector.tensor_tensor(
                                out=H[:, :tl, :],
                                in0=iota_s[:].unsqueeze(1).broadcast_to([128, tl, 128]),
                                in1=grow_s[:, c0 + g_lo : c0 + g_lo + tl]
                                .unsqueeze(2)
                                .broadcast_to([128, tl, 128]),
                                op=mybir.AluOpType.is_equal,
                            )
                        if th:
                            nc.vector.tensor_tensor(
                                out=H[:, tl:T, :],
                                in0=iota_s[:].unsqueeze(1).broadcast_to([128, th, 128]),
                                in1=grow_s[:, c0 + g_hi : c0 + g_hi + th]
                                .unsqueeze(2)
                                .broadcast_to([128, th, 128]),
                                op=mybir.AluOpType.is_equal,
                            )
                        agg_ps = aggpsp.tile([128, 128], f32, name="agg_ps", space="PSUM")
                        gtiles = [g_lo + j for j in range(tl)] + [g_hi + j for j in range(th)]
                        for j, gt in enumerate(gtiles):
                            nc.tensor.matmul(
                                out=agg_ps[:],
                                lhsT=G[:, gt, :],
                                rhs=H[:, j, :],
                                start=(j == 0),
                                stop=(j == T - 1),
                            )
                        agg_sb = aggsbp.tile([128, 128], DT, name="agg_sb")
                        nc.scalar.copy(out=agg_sb[:], in_=agg_ps[:])
                        nc.tensor.matmul(
                            out=out_ps[:],
                            lhsT=bw_s[:, r * 128 : (r + 1) * 128],
                            rhs=agg_sb[:],
                            start=(r == 0),
                            stop=(r == N_REL),
                        )
                    out_sb = outsbp.tile([128, 128], f32, name="out_sb")
                    nc.scalar.copy(out=out_sb[:], in_=out_ps[:])
                    nc.sync.dma_start(out=out_d[:, b * 128 : (b + 1) * 128], in_=out_sb[:])
                rep_ctx.close()

    nc.compile()
    names = {
        "xt": xt_d.tensor.name,
        "bw": bw_d.tensor.name,
        "iota": iota_d.tensor.name,
        "ilo": ilo_d.tensor.name,
        "ihi": ihi_d.tensor.name,
        "grow": grow_d.tensor.name,
        "gw": gw_d.tensor.name,
        "out": out_d.tensor.name,
    }
    return nc, names


def _block_diag_bw(blocks, dt_name):
    """blocks (R+1, nb, bs, bs) -> [128, (R+1)*128] block-diagonal, (in, out)."""
    npdt = _np_dt(dt_name)
    nrel1, nb, bs, _ = blocks.shape
    bw = np.zeros((128, nrel1 * 128), dtype=np.float32)
    for r in range(nrel1):
        for a in range(nb):
            bw[a * bs : (a + 1) * bs, r * 128 + a * bs : r * 128 + (a + 1) * bs] = blocks[r, a]
    return bw.astype(npdt)


def _prep(x, blocks, node_keep_mask, source, target, edge_type, edge_weights, _dt):
    x = np.asarray(x, dtype=np.float32)
    blocks = np.asarray(blocks, dtype=np.float32)
    keep = np.asarray(node_keep_mask).astype(bool)
    source = np.asarray(source).astype(np.int64)
    target = np.asarray(target).astype(np.int64)
    edge_type = np.asarray(edge_type).astype(np.int64)
    edge_weights = np.asarray(edge_weights, dtype=np.float32)

    npdt = _np_dt(_dt)
    src = np.concatenate([source, target])
    dst = np.concatenate([target, source])
    rel = np.concatenate([edge_type, edge_type])
    w = np.concatenate([edge_weights, edge_weights])

    lay, percore = _prepare_layout(src, dst, rel, w, keep)
    ilo_all, ihi_all, grow_all, gw_all = _prepare_core_meta(lay, percore, _dt)

    xt = x.astype(npdt)
    bw = _block_diag_bw(blocks, _dt)
    iota = np.tile(np.arange(128, dtype=np.float32), (128, 1)).astype(npdt)
    data = (xt, bw, iota, ilo_all, ihi_all, grow_all, gw_all)
    return lay, data


def _in_maps(names, data):
    xt, bw, iota, ilo_all, ihi_all, grow_all, gw_all = data
    return [
        {
            names["xt"]: xt,
            names["bw"]: bw,
            names["iota"]: iota,
            names["ilo"]: ilo_all[k],
            names["ihi"]: ihi_all[k],
            names["grow"]: grow_all[k],
            names["gw"]: gw_all[k],
        }
        for k in range(NCORES)
    ]


def _get_program(_dt, lay, repeat=1):
    key = (_dt,) + lay.key() + (repeat,)
    if key not in _cache:
        _cache[key] = _build_program(_dt, lay, repeat)
    return _cache[key]


def kernel(x, blocks, node_keep_mask, source, target, edge_type, edge_weights, _dt="float32"):
    from concourse.bass_utils import run_bass_kernel_spmd

    lay, data = _prep(
        x, blocks, node_keep_mask, source, target, edge_type, edge_weights, _dt
    )
    nc, names = _get_program(_dt, lay)
    res = run_bass_kernel_spmd(nc, _in_maps(names, data), list(range(NCORES)))
    global last_result
    last_result = res
    out = np.concatenate(
        [np.asarray(res.results[k][names["out"]]).T[:SHARD] for k in range(NCORES)],
        axis=0,
    ).astype(np.float32)
    return out


def measure_hw_ns(inputs, _dt="float32", big_rep=257, n_runs=3):
    """Differential HW timing: wall(rep=big) - wall(rep=1) over (big-1) bodies.

    Returns (body_ns, out_of_big_rep_run) -- the second for a correctness
    cross-check (the repeated body recomputes the identical output).
    """
    import time

    from concourse.bass_utils import run_bass_kernel_spmd

    lay, data = _prep(_dt=_dt, **inputs)
    walls = {}
    out_big = None
    for rep in (1, big_rep):
        nc, names = _get_program(_dt, lay, rep)
        maps = _in_maps(names, data)
        res = run_bass_kernel_spmd(nc, maps, list(range(NCORES)))  # warm
        best = float("inf")
        for _ in range(n_runs):
            t0 = time.perf_counter()
            res = run_bass_kernel_spmd(nc, maps, list(range(NCORES)))
            best = min(best, time.perf_counter() - t0)
        walls[rep] = best
        if rep == big_rep:
            out_big = np.concatenate(
                [
                    np.asarray(res.results[k][names["out"]]).T[:SHARD]
                    for k in range(NCORES)
                ],
                axis=0,
            ).astype(np.float32)
    body_ns = (walls[big_rep] - walls[1]) / (big_rep - 1) * 1e9
    print(
        f"wall rep=1: {walls[1] * 1e3:.1f} ms, rep={big_rep}: "
        f"{walls[big_rep] * 1e3:.1f} ms -> body {body_ns:.0f} ns"
    )
    return body_ns, out_big



# revision 2
# speedup vs baseline: 13.0156x; 13.0156x over previous
"""Trainium2 Bass kernel for relational GNN message passing (BlockDecomposition).

Strategy (8 NeuronCores, SPMD, no collectives):
  - Symmetrize edges into 1.6M directed messages + 50k self-loop messages.
  - Shard messages by destination node range (6250 nodes/core); each core
    produces its own output rows.
  - Host sorts each core's messages by (dest bucket of 128 nodes, src half,
    relation). dma_gather int16 indices only address 32768 rows, so each
    bucket's slots are laid out as [low-half region | high-half region]; each
    region is one dma_gather (low from x[0:32768], high from x[32768:]).
    The gather ucode requires every index position to be valid (interior
    skips corrupt), so pad slots gather dummy row 0 and carry weight 0.
  - Per (bucket, rel) group: static tile counts T_LO/T_HI = max over cores
    (same compiled program for all 8 cores; slack slots are dummies).
  - Device, per bucket:
      * two dma_gathers of x[src] rows into G,
      * DVE: G *= w (dummy slots have w = 0),
      * per rel: aggregation matmuls aggT += G_tile^T @ H_tile in PSUM, with
        one-hot H[e, n] = (dst_row[e] == n) built by DVE is_equal vs iota
        (dummy slots have dst_row = 255 -> zero column),
      * transform matmuls outT += BW_r^T @ aggT accumulated over 9 relations,
      * store the outT tile (128 dims x 128 nodes) to HBM.
  - Host transposes/concatenates per-core outputs.
"""

import sys

import numpy as np

sys.path.insert(0, "/opt/trn_rl_repo")

# Problem constants (hardcoded per spec).
N_NODES = 50000
DIM = 128
N_REL = 8  # edge relations; +1 self-loop "relation"
NCORES = 8
SHARD = N_NODES // NCORES  # 6250
NBUCK = (SHARD + 127) // 128  # 49
PADN = NBUCK * 128  # 6272
LO = 32768  # int16-addressable rows per gather table

_PAD_ROW = 255.0  # no is_equal match vs iota 0..127 -> zero one-hot column

_cache = {}
last_result = None  # BassKernelResults of the most recent run (for profiling)


def _np_dt(dt_name):
    if dt_name == "float32":
        return np.float32
    import ml_dtypes

    return np.dtype(getattr(ml_dtypes, dt_name))


class Layout:
    """Static slot layout shared by all cores (max tile counts over cores)."""

    def __init__(self, tlo, thi):
        # tlo/thi: [NBUCK, N_REL+1] tiles per (bucket, rel) per src half
        self.tlo = tlo
        self.thi = thi
        self.nlo = tlo.sum(axis=1)  # lo-region tiles per bucket
        self.nhi = thi.sum(axis=1)
        self.tb = self.nlo + self.nhi  # tiles per bucket
        self.c0 = np.zeros(NBUCK, dtype=np.int64)  # global tile offset
        self.c0[1:] = np.cumsum(self.tb)[:-1]
        self.nt = int(self.tb.sum())  # total slot tiles per core
        # within-bucket tile offsets of each (rel, half) group part
        self.lo_off = np.zeros((NBUCK, N_REL + 1), dtype=np.int64)
        self.lo_off[:, 1:] = np.cumsum(tlo, axis=1)[:, :-1]
        self.hi_off = np.zeros((NBUCK, N_REL + 1), dtype=np.int64)
        self.hi_off[:, 1:] = np.cumsum(thi, axis=1)[:, :-1]
        # idx-array column offsets per bucket (NI/16 columns per bucket)
        self.olo = np.zeros(NBUCK, dtype=np.int64)
        self.olo[1:] = np.cumsum(self.nlo * 8)[:-1]
        self.ohi = np.zeros(NBUCK, dtype=np.int64)
        self.ohi[1:] = np.cumsum(self.nhi * 8)[:-1]
        self.ilo_cols = int((self.nlo * 8).sum())
        self.ihi_cols = int((self.nhi * 8).sum())

    def key(self):
        return (self.tlo.tobytes(), self.thi.tobytes())


def _message_arrays(src, dst, rel, w, keep, k):
    """All directed messages destined to core k, plus its self-loops."""
    m = (dst >= k * SHARD) & (dst < (k + 1) * SHARD)
    n0 = k * SHARD
    s_k = np.concatenate([src[m], np.arange(n0, n0 + SHARD, dtype=np.int64)])
    l_k = np.concatenate([dst[m] - n0, np.arange(SHARD, dtype=np.int64)])
    r_k = np.concatenate([rel[m], np.full(SHARD, N_REL, dtype=np.int64)])
    w_k = np.concatenate([w[m], keep[n0 : n0 + SHARD].astype(np.float32)])
    return s_k, l_k, r_k, w_k


def _prepare_layout(src, dst, rel, w, keep):
    cnt = np.zeros((NCORES, NBUCK, N_REL + 1, 2), dtype=np.int64)
    percore = []
    for k in range(NCORES):
        s_k, l_k, r_k, w_k = _message_arrays(src, dst, rel, w, keep, k)
        half = (s_k >= LO).astype(np.int64)
        bucket = l_k // 128
        np.add.at(cnt[k], (bucket, r_k, half), 1)
        percore.append((s_k, l_k, r_k, w_k, half, bucket))
    tlo = -(-cnt[:, :, :, 0].max(axis=0) // 128)
    thi = -(-cnt[:, :, :, 1].max(axis=0) // 128)
    return Layout(tlo, thi), percore


def _prepare_core_meta(lay, percore, dt_name):
    npdt = _np_dt(dt_name)
    ilo_all, ihi_all, grow_all, gw_all = [], [], [], []
    for k in range(NCORES):
        s_k, l_k, r_k, w_k, half, bucket = percore[k]
        row = l_k % 128
        # sort by src within each (bucket, rel, half) group: monotonic gather
        # addresses per instruction window -> HBM row locality
        order = np.lexsort((s_k, half, r_k, bucket))
        s_k, r_k, w_k, half, bucket, row = (
            a[order] for a in (s_k, r_k, w_k, half, bucket, row)
        )
        g = (bucket * (N_REL + 1) + r_k) * 2 + half
        sizes = np.bincount(g, minlength=NBUCK * (N_REL + 1) * 2)
        starts = np.zeros_like(sizes)
        starts[1:] = np.cumsum(sizes)[:-1]
        rank = np.arange(len(g)) - starts[g]

        # global tile of each message
        t_lo = lay.c0[bucket] + lay.lo_off[bucket, r_k] + rank // 128
        t_hi = lay.c0[bucket] + lay.nlo[bucket] + lay.hi_off[bucket, r_k] + rank // 128
        t = np.where(half == 0, t_lo, t_hi)
        p = rank % 128

        grow = np.full((128, lay.nt), _PAD_ROW, dtype=np.float32)
        gw = np.zeros((128, lay.nt), dtype=np.float32)
        grow[p, t] = row
        gw[p, t] = w_k

        # gather-position i within the bucket's region; idx element i of
        # bucket b lives at [i % 16, region_col_offset_b + i // 16]
        t_in_region = np.where(
            half == 0, t - lay.c0[bucket], t - lay.c0[bucket] - lay.nlo[bucket]
        )
        i_pos = t_in_region * 128 + p
        col_off = np.where(half == 0, lay.olo[bucket], lay.ohi[bucket])
        cols = col_off + i_pos // 16
        prow = i_pos % 16
        ilo = np.zeros((16, lay.ilo_cols), dtype=np.int16)  # dummies -> row 0
        ihi = np.zeros((16, lay.ihi_cols), dtype=np.int16)
        is_lo = half == 0
        ilo[prow[is_lo], cols[is_lo]] = s_k[is_lo].astype(np.int16)
        ihi[prow[~is_lo], cols[~is_lo]] = (s_k[~is_lo] - LO).astype(np.int16)

        ilo_all.append(np.tile(ilo, (8, 1)))
        ihi_all.append(np.tile(ihi, (8, 1)))
        grow_all.append(grow.astype(npdt))
        gw_all.append(gw.astype(npdt))
    return ilo_all, ihi_all, grow_all, gw_all


def _build_program(dt_name, lay, repeat=1):
    """Build + compile the Bass/Tile program. Returns (nc, io_names).

    repeat > 1 wraps the compute in a hardware loop re-running the identical
    body (same output) -- used for differential wall-clock timing.
    """
    from contextlib import ExitStack

    from concourse import bacc, mybir
    import concourse.tile as tile

    DT = getattr(mybir.dt, dt_name)
    f32 = mybir.dt.float32
    i16 = mybir.dt.int16

    nc = bacc.Bacc(None, target_bir_lowering=False, debug=False)

    with tile.TileContext(nc) as tc:
        with tc.tile_pool(name="dram", bufs=1, space="DRAM") as dram:
            xt_d = dram.tile([N_NODES, DIM], DT, kind="ExternalInput", name="xt")
            bw_d = dram.tile([128, (N_REL + 1) * 128], DT, kind="ExternalInput", name="bw")
            iota_d = dram.tile([128, 128], DT, kind="ExternalInput", name="iota")
            ilo_d = dram.tile([128, lay.ilo_cols], i16, kind="ExternalInput", name="ilo")
            ihi_d = dram.tile([128, lay.ihi_cols], i16, kind="ExternalInput", name="ihi")
            grow_d = dram.tile([128, lay.nt], DT, kind="ExternalInput", name="grow")
            gw_d = dram.tile([128, lay.nt], DT, kind="ExternalInput", name="gw")
            out_d = dram.tile([128, PADN], f32, kind="ExternalOutput", name="outT")

            max_tb = int(lay.tb.max())
            with (
                tc.tile_pool(name="const", bufs=1) as constp,
                tc.tile_pool(name="ipool", bufs=3) as ipool,
                tc.tile_pool(name="gpool", bufs=3) as gpool,
                tc.tile_pool(name="hpool", bufs=6) as hpool,
                tc.tile_pool(name="aggsb", bufs=6) as aggsbp,
                tc.tile_pool(name="outsb", bufs=3) as outsbp,
                tc.tile_pool(name="aggps", bufs=4, space="PSUM") as aggpsp,
                tc.tile_pool(name="outps", bufs=2, space="PSUM") as outpsp,
            ):
                iota_s = constp.tile([128, 128], DT)
                bw_s = constp.tile([128, (N_REL + 1) * 128], DT)
                grow_s = constp.tile([128, lay.nt], DT)
                gw_s = constp.tile([128, lay.nt], DT)
                nc.sync.dma_start(out=iota_s[:], in_=iota_d[:])
                nc.sync.dma_start(out=bw_s[:], in_=bw_d[:])
                nc.sync.dma_start(out=grow_s[:], in_=grow_d[:])
                nc.sync.dma_start(out=gw_s[:], in_=gw_d[:])

                rep_ctx = ExitStack()
                if repeat > 1:
                    rep_ctx.enter_context(tc.For_i(0, repeat, 1))
                for b in range(NBUCK):
                    c0 = int(lay.c0[b])
                    nlo = int(lay.nlo[b])
                    nhi = int(lay.nhi[b])
                    tb = nlo + nhi
                    ilo = ipool.tile([128, 8 * max_tb], i16, name="ilo_t")
                    ihi = ipool.tile([128, 8 * max_tb], i16, name="ihi_t")
                    nc.sync.dma_start(
                        out=ilo[:, : 8 * nlo],
                        in_=ilo_d[:, int(lay.olo[b]) : int(lay.olo[b]) + 8 * nlo],
                    )
                    nc.sync.dma_start(
                        out=ihi[:, : 8 * nhi],
                        in_=ihi_d[:, int(lay.ohi[b]) : int(lay.ohi[b]) + 8 * nhi],
                    )
                    G = gpool.tile([128, max_tb, DIM], DT, name="G")
                    nc.gpsimd.dma_gather(
                        G[:, 0:nlo, :],
                        xt_d[0:LO],
                        ilo[:, : 8 * nlo],
                        nlo * 128,
                        nlo * 128,
                        DIM,
                        single_packet=False,
                    )
                    nc.gpsimd.dma_gather(
                        G[:, nlo:tb, :],
                        xt_d[LO:N_NODES],
                        ihi[:, : 8 * nhi],
                        nhi * 128,
                        nhi * 128,
                        DIM,
                        single_packet=False,
                    )
                    # G *= w  (broadcast along features; dummy slots have w=0)
                    nc.vector.tensor_tensor(
                        out=G[:, 0:tb, :],
                        in0=G[:, 0:tb, :],
                        in1=gw_s[:, c0 : c0 + tb].unsqueeze(2).broadcast_to([128, tb, DIM]),
                        op=mybir.AluOpType.mult,
                    )

                    out_ps = outpsp.tile([128, 128], f32, name="out_ps", space="PSUM")
                    for r in range(N_REL + 1):
                        tl = int(lay.tlo[b, r])
                        th = int(lay.thi[b, r])
                        T = tl + th
                        # group's global tile columns within G / grow
                        g_lo = int(lay.lo_off[b, r])
                        g_hi = nlo + int(lay.hi_off[b, r])
                        H = hpool.tile([128, max(T, 1), 128], DT, name="H", tag="H")
                        if tl:
                            nc.vector.tensor_tensor(
                                out=H[:, :tl, :],
                                in0=iota_s[:].unsqueeze(1).broadcast_to([128, tl, 128]),
                                in1=grow_s[:, c0 + g_lo : c0 + g_lo + tl]
                                .unsqueeze(2)
                                .broadcast_to([128, tl, 128]),
                                op=mybir.AluOpType.is_equal,
                            )
                        if th:
                            nc.vector.tensor_tensor(
                                out=H[:, tl:T, :],
                                in0=iota_s[:].unsqueeze(1).broadcast_to([128, th, 128]),
                                in1=grow_s[:, c0 + g_hi : c0 + g_hi + th]
                                .unsqueeze(2)
                                .broadcast_to([128, th, 128]),
                                op=mybir.AluOpType.is_equal,
                            )
                        agg_ps = aggpsp.tile([128, 128], f32, name="agg_ps", space="PSUM")
                        gtiles = [g_lo + j for j in range(tl)] + [g_hi + j for j in range(th)]
                        for j, gt in enumerate(gtiles):
                            nc.tensor.matmul(
                                out=agg_ps[:],
                                lhsT=G[:, gt, :],
                                rhs=H[:, j, :],
                                start=(j == 0),
                                stop=(j == T - 1),
                            )
                        agg_sb = aggsbp.tile([128, 128], DT, name="agg_sb")
                        nc.scalar.copy(out=agg_sb[:], in_=agg_ps[:])
                        nc.tensor.matmul(
                            out=out_ps[:],
                            lhsT=bw_s[:, r * 128 : (r + 1) * 128],
                            rhs=agg_sb[:],
                            start=(r == 0),
                            stop=(r == N_REL),
                        )
                    out_sb = outsbp.tile([128, 128], f32, name="out_sb")
                    nc.scalar.copy(out=out_sb[:], in_=out_ps[:])
                    nc.sync.dma_start(out=out_d[:, b * 128 : (b + 1) * 128], in_=out_sb[:])
                rep_ctx.close()

    nc.compile()
    names = {
        "xt": xt_d.tensor.name,
        "bw": bw_d.tensor.name,
        "iota": iota_d.tensor.name,
        "ilo": ilo_d.tensor.name,
        "ihi": ihi_d.tensor.name,
        "grow": grow_d.tensor.name,
        "gw": gw_d.tensor.name,
        "out": out_d.tensor.name,
    }
    return nc, names


def _block_diag_bw(blocks, dt_name):
    """blocks (R+1, nb, bs, bs) -> [128, (R+1)*128] block-diagonal, (in, out)."""
    npdt = _np_dt(dt_name)
    nrel1, nb, bs, _ = blocks.shape
    bw = np.zeros((128, nrel1 * 128), dtype=np.float32)
    for r in range(nrel1):
        for a in range(nb):
            bw[a * bs : (a + 1) * bs, r * 128 + a * bs : r * 128 + (a + 1) * bs] = blocks[r, a]
    return bw.astype(npdt)


def _prep(x, blocks, node_keep_mask, source, target, edge_type, edge_weights, _dt):
    x = np.asarray(x, dtype=np.float32)
    blocks = np.asarray(blocks, dtype=np.float32)
    keep = np.asarray(node_keep_mask).astype(bool)
    source = np.asarray(source).astype(np.int64)
    target = np.asarray(target).astype(np.int64)
    edge_type = np.asarray(edge_type).astype(np.int64)
    edge_weights = np.asarray(edge_weights, dtype=np.float32)

    npdt = _np_dt(_dt)
    src = np.concatenate([source, target])
    dst = np.concatenate([target, source])
    rel = np.concatenate([edge_type, edge_type])
    w = np.concatenate([edge_weights, edge_weights])

    lay, percore = _prepare_layout(src, dst, rel, w, keep)
    ilo_all, ihi_all, grow_all, gw_all = _prepare_core_meta(lay, percore, _dt)

    xt = x.astype(npdt)
    bw = _block_diag_bw(blocks, _dt)
    iota = np.tile(np.arange(128, dtype=np.float32), (128, 1)).astype(npdt)
    data = (xt, bw, iota, ilo_all, ihi_all, grow_all, gw_all)
    return lay, data


def _in_maps(names, data):
    xt, bw, iota, ilo_all, ihi_all, grow_all, gw_all = data
    return [
        {
            names["xt"]: xt,
            names["bw"]: bw,
            names["iota"]: iota,
            names["ilo"]: ilo_all[k],
            names["ihi"]: ihi_all[k],
            names["grow"]: grow_all[k],
            names["gw"]: gw_all[k],
        }
        for k in range(NCORES)
    ]


def _get_program(_dt, lay, repeat=1):
    key = (_dt,) + lay.key() + (repeat,)
    if key not in _cache:
        _cache[key] = _build_program(_dt, lay, repeat)
    return _cache[key]


def kernel(x, blocks, node_keep_mask, source, target, edge_type, edge_weights, _dt="float32"):
    from concourse.bass_utils import run_bass_kernel_spmd

    lay, data = _prep(
        x, blocks, node_keep_mask, source, target, edge_type, edge_weights, _dt
    )
    nc, names = _get_program(_dt, lay)
    res = run_bass_kernel_spmd(nc, _in_maps(names, data), list(range(NCORES)))
    global last_result
    last_result = res
    out = np.concatenate(
        [np.asarray(res.results[k][names["out"]]).T[:SHARD] for k in range(NCORES)],
        axis=0,
    ).astype(np.float32)
    return out


def measure_hw_ns(inputs, _dt="float32", big_rep=1025, n_runs=4):
    """Differential HW timing: wall(rep=big) - wall(rep=1) over (big-1) bodies.

    Returns (body_ns, out_of_big_rep_run) -- the second for a correctness
    cross-check (the repeated body recomputes the identical output).
    """
    import time

    from concourse.bass_utils import run_bass_kernel_spmd

    lay, data = _prep(_dt=_dt, **inputs)
    walls = {}
    out_big = None
    for rep in (1, big_rep):
        nc, names = _get_program(_dt, lay, rep)
        maps = _in_maps(names, data)
        res = run_bass_kernel_spmd(nc, maps, list(range(NCORES)))  # warm
        best = float("inf")
        for _ in range(n_runs):
            t0 = time.perf_counter()
            res = run_bass_kernel_spmd(nc, maps, list(range(NCORES)))
            best = min(best, time.perf_counter() - t0)
        walls[rep] = best
        if rep == big_rep:
            out_big = np.concatenate(
                [
                    np.asarray(res.results[k][names["out"]]).T[:SHARD]
                    for k in range(NCORES)
                ],
                axis=0,
            ).astype(np.float32)
    body_ns = (walls[big_rep] - walls[1]) / (big_rep - 1) * 1e9
    print(
        f"wall rep=1: {walls[1] * 1e3:.1f} ms, rep={big_rep}: "
        f"{walls[big_rep] * 1e3:.1f} ms -> body {body_ns:.0f} ns"
    )
    return body_ns, out_big

